# revision 46
# baseline (speedup 1.0000x reference)
"""Trainium2 Bass kernel for the multimodal connector problem.

Computes, for each batch sample (one NeuronCore per sample):
  projected = visual_features @ W_proj + b_proj          (PE matmul)
  text_emb  = embedding_table[texts]                     (indirect-DMA gather)
  fused     = take_along_axis([text_emb; projected], idx) * valid
  mask      = nested block-causal attention mask         (iota + compare)

The ragged plan (gather indices / chunk ids) is integer-only host work,
mirroring the reference, and is baked into the instruction stream as
static DMA segment lists.  Every mask row is a prefix of ones of length
K[i], so the mask is generated on-device as `iota_j < K[p]` row tiles and
only the nonzero column prefix is written (the runner zero-fills outputs).
"""

import math

import numpy as np

P = 128
NSPLIT = 512  # matmul moving-operand free dim (fp32 max)

# Results of the most recent device run (for the dev harness to inspect).
LAST_RESULTS = None

# Extra kwargs for bass.Bass() (dev experimentation).
_BASS_KW = {}

# Apply the 1-wait-per-instruction legalization (hardware needs it;
# CoreSim rejects the synthetic EventSemaphore instructions).
_SPLIT_WAITS = True


# ---------------------------------------------------------------------------
# Host-side integer planning (mirrors reference._plan)
# ---------------------------------------------------------------------------

def _plan_lists(texts_np, image_token_id, n_visual):
    Bn, Sn = texts_np.shape
    idx_list, cid_list, vis_list = [], [], []
    for b in range(Bn):
        pos = np.nonzero(texts_np[b] == image_token_id)[0]
        n = len(pos)
        if n == 0:
            idx_list.append(np.arange(Sn))
            cid_list.append(np.zeros(Sn, np.int64))
            vis_list.append(np.zeros(Sn, bool))
            continue
        per = n_visual // n
        text_chunks = []
        start = 0
        for p in pos:
            if p > start:
                text_chunks.append((start, int(p)))
            start = int(p) + 1
        if start < Sn:
            text_chunks.append((start, Sn))
        idx, cid, vis = [], [], []
        c = 0
        for i in range(n):
            if i < len(text_chunks):
                s, e = text_chunks[i]
                idx.append(np.arange(s, e))
                cid.append(np.full(e - s, c))
                vis.append(np.zeros(e - s, bool))
                c += 1
            idx.append(Sn + np.arange(i * per, (i + 1) * per))
            cid.append(np.full(per, c))
            vis.append(np.ones(per, bool))
            c += 1
        if n < len(text_chunks):
            s, e = text_chunks[-1]
            idx.append(np.arange(s, e))
            cid.append(np.full(e - s, c))
            vis.append(np.zeros(e - s, bool))
        idx_list.append(np.concatenate(idx))
        cid_list.append(np.concatenate(cid))
        vis_list.append(np.concatenate(vis))
    return idx_list, cid_list, vis_list


def _runs(flags):
    """Maximal runs of True in a bool vector -> list of (start, length)."""
    out = []
    i, n = 0, len(flags)
    while i < n:
        if flags[i]:
            j = i
            while j < n and flags[j]:
                j += 1
            out.append((i, j - i))
            i = j
        else:
            i += 1
    return out


def _sample_meta(texts_row, idx, cid, vis, S, L):
    """Per-sample static structure + runtime vectors."""
    l = len(idx)
    tvis = np.asarray(vis, bool)
    # Compact text-token rows in fused order.
    text_pos = idx[~tvis]
    tok_rows = texts_row[text_pos].astype(np.int32)
    n_text = len(tok_rows)

    # Text segments: (compact_start, fused_start, length)
    text_segs = []
    c = 0
    for s, ln in _runs(~tvis):
        text_segs.append((c, s, ln))
        c += ln
    # Visual segments: (proj_start, fused_start, length)
    vis_segs = []
    for s, ln in _runs(tvis):
        prows = idx[s:s + ln] - S
        assert np.all(np.diff(prows) == 1), "visual rows not contiguous"
        vis_segs.append((int(prows[0]), s, ln))

    # Per-row prefix length K of the mask.
    K = np.zeros(l, np.int64)
    ar = np.arange(l)
    K[~tvis] = ar[~tvis] + 1
    if tvis.any():
        cida = np.asarray(cid)
        # end index (exclusive) of each chunk id
        ends = {}
        for i in range(l):
            ends[int(cida[i])] = i + 1
        for i in np.nonzero(tvis)[0]:
            K[i] = ends[int(cida[i])]

    K_full = np.zeros(L, np.int64)
    K_full[:l] = K
    return {
        "l": l,
        "tok_rows": tok_rows,
        "n_text": n_text,
        "text_segs": tuple(text_segs),
        "vis_segs": tuple(vis_segs),
        "K": K_full,
    }


def _tile_segments(segs, tile_lo, tile_hi, src_key):
    """Intersect (src_start, fused_start, length) segments with the
    source-coordinate range [tile_lo, tile_hi) -> (sbuf_row, fused_row, n)."""
    out = []
    for src0, dst0, ln in segs:
        lo = max(src0, tile_lo)
        hi = min(src0 + ln, tile_hi)
        if lo < hi:
            out.append((lo - tile_lo, dst0 + (lo - src0), hi - lo))
    return out


# ---------------------------------------------------------------------------
# Bass program builder (one structure -> one SPMD program)
# ---------------------------------------------------------------------------

def _split_excess_waits(nc):
    """This compiler build allows a single sync-wait slot per instruction.
    Move all but the last wait of any instruction onto standalone
    EventSemaphore instructions (what wait_ge emits) on the same engine,
    immediately before it — semantically identical, encodable."""
    import concourse.mybir as mybir

    ctr = 0
    for blk in nc.m.functions[0].blocks:
        new_list = []
        changed = False
        for inst in blk.instructions:
            si = inst.sync_info
            if si is not None and len(si.on_wait) > 1:
                waits = list(si.on_wait)
                for w in waits[:-1]:
                    ev = mybir.InstEventSemaphore(
                        name=f"I-wsplit-{ctr}", ins=[], outs=[]
                    )
                    ctr += 1
                    ev.engine = inst.engine
                    ev.sync_info = mybir.SyncInfo(on_wait=[w], on_update=[])
                    new_list.append(ev)
                inst.sync_info = mybir.SyncInfo(
                    on_wait=[waits[-1]], on_update=list(si.on_update)
                )
                changed = True
            new_list.append(inst)
        if changed:
            blk.instructions = new_list
    return nc


def _build_nc(struct):
    import concourse.bass as bass
    import concourse.mybir as mybir
    import concourse.tile as tile

    (V, D, DV, Nv, L, n_text, text_segs, vis_segs, kmax_list, l) = struct
    f32 = mybir.dt.float32
    i32 = mybir.dt.int32

    KC = DV // P               # contraction chunks (1152/128 = 9)
    MT = Nv // P               # projected row tiles (2304/128 = 18)
    NT = D // NSPLIT           # output col chunks (1024/512 = 2)
    n_ttiles = math.ceil(n_text / P) if n_text else 0
    n_mtiles = math.ceil(L / P)

    # which projected-row tiles are actually consumed
    used_m = []
    for m in range(MT):
        if _tile_segments(vis_segs, m * P, (m + 1) * P, "vis"):
            used_m.append(m)

    f32r = mybir.dt.float32r  # same 4-byte storage as f32, 4x PE throughput
    nc = bass.Bass("TRN2", **_BASS_KW)
    emb_d = nc.dram_tensor("emb", [V, D], f32, kind="ExternalInput")
    xT_d = nc.dram_tensor("xt", [DV, Nv], f32r, kind="ExternalInput")
    w_d = nc.dram_tensor("w", [DV, D], f32r, kind="ExternalInput")
    b_d = nc.dram_tensor("bias", [1, D], f32r, kind="ExternalInput")
    ones_d = nc.dram_tensor("ones", [1, P], f32r, kind="ExternalInput")
    tok_d = (nc.dram_tensor("tok", [P, n_ttiles], i32, kind="ExternalInput")
             if n_ttiles else None)
    k_d = nc.dram_tensor("kvec", [P, n_mtiles], f32, kind="ExternalInput")
    fused_d = nc.dram_tensor("fused", [L, D], f32, kind="ExternalOutput")
    mask_d = nc.dram_tensor("maskout", [L, L], f32, kind="ExternalOutput")

    # SP ring is dedicated to feeding the PE (xT/const loads); all output
    # writes go to the ACT ring (+ gpsimd for text) so a queued 2MB mask
    # write can never stall the next xT load and de-warm the PE.
    def wr_eng():
        return nc.scalar

    with tile.TileContext(nc) as tc:
        with (
            tc.tile_pool(name="const", bufs=1) as cpool,
            tc.tile_pool(name="work", bufs=3) as wpool,
            tc.tile_pool(name="maskp", bufs=4) as mpool,
            tc.tile_pool(name="psum", bufs=4, space="PSUM") as ppool,
        ):
            # ---- constants -------------------------------------------------
            w_sb = cpool.tile([P, KC, D], f32r)
            nc.sync.dma_start(
                out=w_sb[:], in_=w_d[:].rearrange("(j p) n -> p j n", p=P)
            )
            b_sb = cpool.tile([1, D], f32r)
            nc.sync.dma_start(out=b_sb[:], in_=b_d[:])
            ones_sb = cpool.tile([1, P], f32r)
            nc.sync.dma_start(out=ones_sb[:], in_=ones_d[:])
            if n_ttiles:
                tok_sb = cpool.tile([P, n_ttiles], i32)
                nc.sync.dma_start(out=tok_sb[:], in_=tok_d[:])
            k_sb = cpool.tile([P, n_mtiles], f32)
            nc.sync.dma_start(out=k_sb[:], in_=k_d[:])
            iota_f = cpool.tile([P, L], f32)
            nc.gpsimd.iota(
                iota_f[:], pattern=[[1, L]], base=0, channel_multiplier=0,
                allow_small_or_imprecise_dtypes=True,
            )

            # Several TRN2 instruction encodings (PE LDW, DVE TensorScalarPtr)
            # hold only ONE sync-wait slot, so each engine must observe every
            # producer semaphore via single-dependency ops before the real
            # work references several tensors at once.
            # (fp32r matmuls need an even innermost free-dim count -> 2-wide)
            trash_ps = ppool.tile([2, 2], f32, tag="trash", bufs=1)
            for prime in (w_sb[0:1, 0, 0:2], b_sb[0:1, 0:2], ones_sb[0:1, 0:2]):
                nc.tensor.matmul(
                    out=trash_ps[:], lhsT=prime, rhs=prime,
                    start=True, stop=True,
                )
            trash_sb = cpool.tile([1, 1], f32)

            # DVE observes the iota/kvec producers first via one-wait reads
            # (TensorScalarPtr has a single sync-wait slot too).
            prime_k = cpool.tile([P, 1], f32)
            nc.vector.tensor_copy(out=prime_k[:], in_=k_sb[:, :1])
            prime_i = cpool.tile([P, 1], f32)
            nc.vector.tensor_copy(out=prime_i[:], in_=iota_f[:, :1])

            xT_r = xT_d[:].rearrange("(j p) m -> p j m", p=P)

            def emit_visual(m):
                # xT loads stay on the SP ring: on the gpsimd queue they
                # serialize behind gathers/text-writes and starve the PE
                xT_sb = wpool.tile([P, KC, P], f32r, tag="xt", bufs=5,
                                   name="xT_sb")
                nc.sync.dma_start(
                    out=xT_sb[:], in_=xT_r[:, :, m * P:(m + 1) * P]
                )
                # PE observes this xT tile's DMA via a tiny matmul (1 wait)
                nc.tensor.matmul(
                    out=trash_ps[:], lhsT=xT_sb[0:1, 0, 0:2],
                    rhs=xT_sb[0:1, 0, 0:2], start=True, stop=True,
                )
                out_sb = wpool.tile([P, D], f32, tag="mout", bufs=4,
                                    name="out_sb")
                for n in range(NT):
                    ps = ppool.tile([P, NSPLIT], f32, tag="ps", bufs=6,
                                    name="ps")
                    for j in range(KC):
                        # float32r streams 1 row/cycle at N>=256 (fp32: 4)
                        nc.tensor.matmul(
                            out=ps[:],
                            lhsT=xT_sb[:, j, :],
                            rhs=w_sb[:, j, n * NSPLIT:(n + 1) * NSPLIT],
                            start=(j == 0),
                            stop=False,
                        )
                    nc.tensor.matmul(
                        out=ps[:],
                        lhsT=ones_sb[:1, :],
                        rhs=b_sb[:1, n * NSPLIT:(n + 1) * NSPLIT],
                        start=False,
                        stop=True,
                    )
                    # DVE observes the matmul-group completion (1 wait)
                    nc.vector.tensor_copy(out=trash_sb[:], in_=ps[0:1, 0:1])
                    nc.vector.tensor_copy(
                        out=out_sb[:, n * NSPLIT:(n + 1) * NSPLIT], in_=ps[:]
                    )
                for sbr, drow, nrows in _tile_segments(
                    vis_segs, m * P, (m + 1) * P, "vis"
                ):
                    wr_eng().dma_start(
                        out=fused_d[drow:drow + nrows, :],
                        in_=out_sb[sbr:sbr + nrows, :],
                    )

            def emit_text(t):
                g_sb = wpool.tile([P, D], f32, tag="gath", name="g_sb")
                nc.gpsimd.indirect_dma_start(
                    out=g_sb[:],
                    out_offset=None,
                    in_=emb_d[:],
                    in_offset=bass.IndirectOffsetOnAxis(
                        ap=tok_sb[:, t:t + 1], axis=0
                    ),
                )
                for sbr, drow, nrows in _tile_segments(
                    text_segs, t * P, (t + 1) * P, "text"
                ):
                    # text writes reach 2MB - keep them off the PE-feeding
                    # SP ring (starves xT loads); gpsimd absorbs them
                    nc.gpsimd.dma_start(
                        out=fused_d[drow:drow + nrows, :],
                        in_=g_sb[sbr:sbr + nrows, :],
                    )

            def emit_mask(t, eng=None):
                kmax = kmax_list[t]
                if kmax == 0:
                    return
                rows = min(P, L - t * P)
                m_sb = mpool.tile([P, L], f32, tag="m", bufs=4, name="m_sb")
                nc.vector.tensor_scalar(
                    out=m_sb[:, :kmax],
                    in0=iota_f[:, :kmax],
                    scalar1=k_sb[:, t:t + 1],
                    scalar2=None,
                    op0=mybir.AluOpType.is_lt,
                )
                if t >= n_mtiles - 8 and kmax > 2048:
                    # tail tiles are the largest and would drain on the ACT
                    # ring alone (SP is idle by then) - split columns across
                    # both rings so the two halves stream in parallel
                    kh = kmax // 2
                    nc.scalar.dma_start(
                        out=mask_d[t * P:t * P + rows, 0:kh],
                        in_=m_sb[:rows, :kh],
                    )
                    nc.sync.dma_start(
                        out=mask_d[t * P:t * P + rows, kh:kmax],
                        in_=m_sb[:rows, kh:kmax],
                    )
                else:
                    (eng or wr_eng()).dma_start(
                        out=mask_d[t * P:t * P + rows, 0:kmax],
                        in_=m_sb[:rows, :kmax],
                    )

            # Interleave the three streams so every DMA queue is fed from
            # t=0. Natural mask order measured fastest: size-reordered or
            # ring-split variants all disturbed the static schedule.
            steps = max(len(used_m), n_ttiles, 1)
            mask_emitted = 0
            for step in range(steps):
                if step < len(used_m):
                    emit_visual(used_m[step])
                if step < n_ttiles:
                    emit_text(step)
                target = ((step + 1) * n_mtiles + steps - 1) // steps
                while mask_emitted < min(target, n_mtiles):
                    emit_mask(mask_emitted)
                    mask_emitted += 1
            while mask_emitted < n_mtiles:
                emit_mask(mask_emitted)
                mask_emitted += 1

            # ---- zero-fill padded fused rows (ragged batch only) -----------
            if l < L:
                z_sb = cpool.tile([P, D], f32)
                nc.vector.memset(z_sb[:], 0.0)
                r = l
                while r < L:
                    nrows = min(P, L - r)
                    wr_eng().dma_start(
                        out=fused_d[r:r + nrows, :], in_=z_sb[:nrows, :]
                    )
                    r += nrows

    return _split_excess_waits(nc) if _SPLIT_WAITS else nc


# ---------------------------------------------------------------------------
# kernel() entry point
# ---------------------------------------------------------------------------

def _run_group(nc, in_maps):
    from concourse.bass_utils import run_bass_kernel_spmd

    global LAST_RESULTS
    res = run_bass_kernel_spmd(nc, in_maps, core_ids=list(range(len(in_maps))))
    LAST_RESULTS = res
    return res.results


# overridable for simulator-based testing
_EXECUTOR = _run_group


def kernel(**inputs):
    visual = np.ascontiguousarray(
        np.asarray(inputs["visual_features"], dtype=np.float32)
    )
    emb = np.ascontiguousarray(
        np.asarray(inputs["embedding_table"], dtype=np.float32)
    )
    W = np.ascontiguousarray(np.asarray(inputs["W_proj"], dtype=np.float32))
    bias = np.ascontiguousarray(
        np.asarray(inputs["b_proj"], dtype=np.float32).reshape(1, -1)
    )
    texts = np.asarray(inputs["texts"]).astype(np.int64)
    itid = int(np.asarray(inputs["image_token_id"]))

    B, S = texts.shape
    _, Nv, DV = visual.shape
    V, D = emb.shape
    assert DV % P == 0 and Nv % P == 0 and D % NSPLIT == 0

    idx_list, cid_list, vis_list = _plan_lists(texts, itid, Nv)
    L = max(len(x) for x in idx_list)
    n_mtiles = math.ceil(L / P)

    metas = [
        _sample_meta(texts[b], idx_list[b], cid_list[b], vis_list[b], S, L)
        for b in range(B)
    ]

    # Group samples by program structure (identical for the standard input).
    groups = {}
    for b, meta in enumerate(metas):
        kmax_list = tuple(
            int(meta["K"][t * P: min((t + 1) * P, L)].max())
            for t in range(n_mtiles)
        )
        key = (V, D, DV, Nv, L, meta["n_text"], meta["text_segs"],
               meta["vis_segs"], kmax_list, meta["l"])
        groups.setdefault(key, []).append(b)

    fused_out = np.zeros((B, L, D), np.float32)
    mask_out = np.zeros((B, 1, L, L), np.float32)

    for struct, bs in groups.items():
        nc = _build_nc(struct)
        n_text = struct[5]
        n_ttiles = math.ceil(n_text / P) if n_text else 0
        in_maps = []
        for b in bs:
            meta = metas[b]
            im = {
                "emb": emb,
                "xt": np.ascontiguousarray(visual[b].T),
                "w": W,
                "bias": bias,
                "ones": np.ones((1, P), np.float32),
                "kvec": np.ascontiguousarray(
                    np.pad(
                        meta["K"].astype(np.float32),
                        (0, n_mtiles * P - L),
                    ).reshape(n_mtiles, P).T
                ),
            }
            if n_ttiles:
                im["tok"] = np.ascontiguousarray(
                    np.pad(meta["tok_rows"], (0, n_ttiles * P - n_text))
                    .reshape(n_ttiles, P).T.astype(np.int32)
                )
            in_maps.append(im)
        results = _EXECUTOR(nc, in_maps)
        for b, res in zip(bs, results):
            fused_out[b] = res["fused"]
            mask_out[b, 0] = res["maskout"]

    return fused_out, mask_out


# revision 50
# speedup vs baseline: 1.4504x; 1.4504x over previous
"""Trainium2 Bass kernel for the multimodal connector problem.

Computes, for each batch sample (one NeuronCore per sample):
  projected = visual_features @ W_proj + b_proj          (PE matmul)
  text_emb  = embedding_table[texts]                     (indirect-DMA gather)
  fused     = take_along_axis([text_emb; projected], idx) * valid
  mask      = nested block-causal attention mask         (iota + compare)

The ragged plan (gather indices / chunk ids) is integer-only host work,
mirroring the reference, and is baked into the instruction stream as
static DMA segment lists.  Every mask row is a prefix of ones of length
K[i], so the mask is generated on-device as `iota_j < K[p]` row tiles and
only the nonzero column prefix is written (the runner zero-fills outputs).
"""

import math

import numpy as np

P = 128
NSPLIT = 512  # matmul moving-operand free dim (fp32 max)

# Results of the most recent device run (for the dev harness to inspect).
LAST_RESULTS = None

# Extra kwargs for bass.Bass() (dev experimentation).
_BASS_KW = {}

# Apply the 1-wait-per-instruction legalization (hardware needs it;
# CoreSim rejects the synthetic EventSemaphore instructions).
_SPLIT_WAITS = True


# ---------------------------------------------------------------------------
# Host-side integer planning (mirrors reference._plan)
# ---------------------------------------------------------------------------

def _plan_lists(texts_np, image_token_id, n_visual):
    Bn, Sn = texts_np.shape
    idx_list, cid_list, vis_list = [], [], []
    for b in range(Bn):
        pos = np.nonzero(texts_np[b] == image_token_id)[0]
        n = len(pos)
        if n == 0:
            idx_list.append(np.arange(Sn))
            cid_list.append(np.zeros(Sn, np.int64))
            vis_list.append(np.zeros(Sn, bool))
            continue
        per = n_visual // n
        text_chunks = []
        start = 0
        for p in pos:
            if p > start:
                text_chunks.append((start, int(p)))
            start = int(p) + 1
        if start < Sn:
            text_chunks.append((start, Sn))
        idx, cid, vis = [], [], []
        c = 0
        for i in range(n):
            if i < len(text_chunks):
                s, e = text_chunks[i]
                idx.append(np.arange(s, e))
                cid.append(np.full(e - s, c))
                vis.append(np.zeros(e - s, bool))
                c += 1
            idx.append(Sn + np.arange(i * per, (i + 1) * per))
            cid.append(np.full(per, c))
            vis.append(np.ones(per, bool))
            c += 1
        if n < len(text_chunks):
            s, e = text_chunks[-1]
            idx.append(np.arange(s, e))
            cid.append(np.full(e - s, c))
            vis.append(np.zeros(e - s, bool))
        idx_list.append(np.concatenate(idx))
        cid_list.append(np.concatenate(cid))
        vis_list.append(np.concatenate(vis))
    return idx_list, cid_list, vis_list


def _runs(flags):
    """Maximal runs of True in a bool vector -> list of (start, length)."""
    out = []
    i, n = 0, len(flags)
    while i < n:
        if flags[i]:
            j = i
            while j < n and flags[j]:
                j += 1
            out.append((i, j - i))
            i = j
        else:
            i += 1
    return out


def _sample_meta(texts_row, idx, cid, vis, S, L):
    """Per-sample static structure + runtime vectors."""
    l = len(idx)
    tvis = np.asarray(vis, bool)
    # Compact text-token rows in fused order.
    text_pos = idx[~tvis]
    tok_rows = texts_row[text_pos].astype(np.int32)
    n_text = len(tok_rows)

    # Text segments: (compact_start, fused_start, length)
    text_segs = []
    c = 0
    for s, ln in _runs(~tvis):
        text_segs.append((c, s, ln))
        c += ln
    # Visual segments: (proj_start, fused_start, length)
    vis_segs = []
    for s, ln in _runs(tvis):
        prows = idx[s:s + ln] - S
        assert np.all(np.diff(prows) == 1), "visual rows not contiguous"
        vis_segs.append((int(prows[0]), s, ln))

    # Per-row prefix length K of the mask.
    K = np.zeros(l, np.int64)
    ar = np.arange(l)
    K[~tvis] = ar[~tvis] + 1
    if tvis.any():
        cida = np.asarray(cid)
        # end index (exclusive) of each chunk id
        ends = {}
        for i in range(l):
            ends[int(cida[i])] = i + 1
        for i in np.nonzero(tvis)[0]:
            K[i] = ends[int(cida[i])]

    K_full = np.zeros(L, np.int64)
    K_full[:l] = K
    return {
        "l": l,
        "tok_rows": tok_rows,
        "n_text": n_text,
        "text_segs": tuple(text_segs),
        "vis_segs": tuple(vis_segs),
        "K": K_full,
    }


def _tile_segments(segs, tile_lo, tile_hi, src_key):
    """Intersect (src_start, fused_start, length) segments with the
    source-coordinate range [tile_lo, tile_hi) -> (sbuf_row, fused_row, n)."""
    out = []
    for src0, dst0, ln in segs:
        lo = max(src0, tile_lo)
        hi = min(src0 + ln, tile_hi)
        if lo < hi:
            out.append((lo - tile_lo, dst0 + (lo - src0), hi - lo))
    return out


# ---------------------------------------------------------------------------
# Bass program builder (one structure -> one SPMD program)
# ---------------------------------------------------------------------------

def _split_excess_waits(nc):
    """This compiler build allows a single sync-wait slot per instruction.
    Move all but the last wait of any instruction onto standalone
    EventSemaphore instructions (what wait_ge emits) on the same engine,
    immediately before it — semantically identical, encodable."""
    import concourse.mybir as mybir

    ctr = 0
    for blk in nc.m.functions[0].blocks:
        new_list = []
        changed = False
        for inst in blk.instructions:
            si = inst.sync_info
            if si is not None and len(si.on_wait) > 1:
                waits = list(si.on_wait)
                for w in waits[:-1]:
                    ev = mybir.InstEventSemaphore(
                        name=f"I-wsplit-{ctr}", ins=[], outs=[]
                    )
                    ctr += 1
                    ev.engine = inst.engine
                    ev.sync_info = mybir.SyncInfo(on_wait=[w], on_update=[])
                    new_list.append(ev)
                inst.sync_info = mybir.SyncInfo(
                    on_wait=[waits[-1]], on_update=list(si.on_update)
                )
                changed = True
            new_list.append(inst)
        if changed:
            blk.instructions = new_list
    return nc


def _build_nc(struct):
    import concourse.bass as bass
    import concourse.mybir as mybir
    import concourse.tile as tile

    (V, D, DV, Nv, L, n_text, text_segs, vis_segs, kmax_list, l) = struct
    f32 = mybir.dt.float32
    i32 = mybir.dt.int32

    KC = DV // P               # contraction chunks (1152/128 = 9)
    MT = Nv // P               # projected row tiles (2304/128 = 18)
    NT = D // NSPLIT           # output col chunks (1024/512 = 2)
    n_ttiles = math.ceil(n_text / P) if n_text else 0
    n_mtiles = math.ceil(L / P)

    # which projected-row tiles are actually consumed
    used_m = []
    for m in range(MT):
        if _tile_segments(vis_segs, m * P, (m + 1) * P, "vis"):
            used_m.append(m)

    f32r = mybir.dt.float32r  # same 4-byte storage as f32, 4x PE throughput
    nc = bass.Bass("TRN2", **_BASS_KW)
    emb_d = nc.dram_tensor("emb", [V, D], f32, kind="ExternalInput")
    xT_d = nc.dram_tensor("xt", [DV, Nv], f32r, kind="ExternalInput")
    w_d = nc.dram_tensor("w", [DV, D], f32r, kind="ExternalInput")
    b_d = nc.dram_tensor("bias", [1, D], f32r, kind="ExternalInput")
    ones_d = nc.dram_tensor("ones", [1, P], f32r, kind="ExternalInput")
    tok_d = (nc.dram_tensor("tok", [P, n_ttiles], i32, kind="ExternalInput")
             if n_ttiles else None)
    k_d = nc.dram_tensor("kvec", [P, n_mtiles], f32, kind="ExternalInput")
    fused_d = nc.dram_tensor("fused", [L, D], f32, kind="ExternalOutput")
    # mask values are 0/1 - write uint8 on device (4x fewer HBM bytes on
    # the dominant stream); the host upcasts to f32 exactly after retrieval
    mask_d = nc.dram_tensor("maskout", [L, L], mybir.dt.uint8,
                            kind="ExternalOutput")

    # SP ring is dedicated to feeding the PE (xT/const loads); all output
    # writes go to the ACT ring (+ gpsimd for text) so a queued 2MB mask
    # write can never stall the next xT load and de-warm the PE.
    def wr_eng():
        return nc.scalar

    with tile.TileContext(nc) as tc:
        with (
            tc.tile_pool(name="const", bufs=1) as cpool,
            tc.tile_pool(name="work", bufs=3) as wpool,
            tc.tile_pool(name="maskp", bufs=4) as mpool,
            tc.tile_pool(name="psum", bufs=4, space="PSUM") as ppool,
        ):
            # ---- constants -------------------------------------------------
            w_sb = cpool.tile([P, KC, D], f32r)
            nc.sync.dma_start(
                out=w_sb[:], in_=w_d[:].rearrange("(j p) n -> p j n", p=P)
            )
            b_sb = cpool.tile([1, D], f32r)
            nc.sync.dma_start(out=b_sb[:], in_=b_d[:])
            ones_sb = cpool.tile([1, P], f32r)
            nc.sync.dma_start(out=ones_sb[:], in_=ones_d[:])
            if n_ttiles:
                tok_sb = cpool.tile([P, n_ttiles], i32)
                nc.sync.dma_start(out=tok_sb[:], in_=tok_d[:])
            k_sb = cpool.tile([P, n_mtiles], f32)
            nc.sync.dma_start(out=k_sb[:], in_=k_d[:])
            iota_f = cpool.tile([P, L], f32)
            nc.gpsimd.iota(
                iota_f[:], pattern=[[1, L]], base=0, channel_multiplier=0,
                allow_small_or_imprecise_dtypes=True,
            )

            # Several TRN2 instruction encodings (PE LDW, DVE TensorScalarPtr)
            # hold only ONE sync-wait slot, so each engine must observe every
            # producer semaphore via single-dependency ops before the real
            # work references several tensors at once.
            # (fp32r matmuls need an even innermost free-dim count -> 2-wide)
            trash_ps = ppool.tile([2, 2], f32, tag="trash", bufs=1)
            for prime in (w_sb[0:1, 0, 0:2], b_sb[0:1, 0:2], ones_sb[0:1, 0:2]):
                nc.tensor.matmul(
                    out=trash_ps[:], lhsT=prime, rhs=prime,
                    start=True, stop=True,
                )
            trash_sb = cpool.tile([1, 1], f32)

            # DVE observes the iota/kvec producers first via one-wait reads
            # (TensorScalarPtr has a single sync-wait slot too).
            prime_k = cpool.tile([P, 1], f32)
            nc.vector.tensor_copy(out=prime_k[:], in_=k_sb[:, :1])
            prime_i = cpool.tile([P, 1], f32)
            nc.vector.tensor_copy(out=prime_i[:], in_=iota_f[:, :1])

            xT_r = xT_d[:].rearrange("(j p) m -> p j m", p=P)

            def emit_visual(m):
                # xT loads stay on the SP ring: on the gpsimd queue they
                # serialize behind gathers/text-writes and starve the PE
                xT_sb = wpool.tile([P, KC, P], f32r, tag="xt", bufs=5,
                                   name="xT_sb")
                nc.sync.dma_start(
                    out=xT_sb[:], in_=xT_r[:, :, m * P:(m + 1) * P]
                )
                # PE observes this xT tile's DMA via a tiny matmul (1 wait)
                nc.tensor.matmul(
                    out=trash_ps[:], lhsT=xT_sb[0:1, 0, 0:2],
                    rhs=xT_sb[0:1, 0, 0:2], start=True, stop=True,
                )
                out_sb = wpool.tile([P, D], f32, tag="mout", bufs=4,
                                    name="out_sb")
                for n in range(NT):
                    ps = ppool.tile([P, NSPLIT], f32, tag="ps", bufs=6,
                                    name="ps")
                    for j in range(KC):
                        # float32r streams 1 row/cycle at N>=256 (fp32: 4)
                        nc.tensor.matmul(
                            out=ps[:],
                            lhsT=xT_sb[:, j, :],
                            rhs=w_sb[:, j, n * NSPLIT:(n + 1) * NSPLIT],
                            start=(j == 0),
                            stop=False,
                        )
                    nc.tensor.matmul(
                        out=ps[:],
                        lhsT=ones_sb[:1, :],
                        rhs=b_sb[:1, n * NSPLIT:(n + 1) * NSPLIT],
                        start=False,
                        stop=True,
                    )
                    # DVE observes the matmul-group completion (1 wait)
                    nc.vector.tensor_copy(out=trash_sb[:], in_=ps[0:1, 0:1])
                    nc.vector.tensor_copy(
                        out=out_sb[:, n * NSPLIT:(n + 1) * NSPLIT], in_=ps[:]
                    )
                for sbr, drow, nrows in _tile_segments(
                    vis_segs, m * P, (m + 1) * P, "vis"
                ):
                    wr_eng().dma_start(
                        out=fused_d[drow:drow + nrows, :],
                        in_=out_sb[sbr:sbr + nrows, :],
                    )

            def emit_text(t):
                g_sb = wpool.tile([P, D], f32, tag="gath", name="g_sb")
                nc.gpsimd.indirect_dma_start(
                    out=g_sb[:],
                    out_offset=None,
                    in_=emb_d[:],
                    in_offset=bass.IndirectOffsetOnAxis(
                        ap=tok_sb[:, t:t + 1], axis=0
                    ),
                )
                for sbr, drow, nrows in _tile_segments(
                    text_segs, t * P, (t + 1) * P, "text"
                ):
                    # text writes reach 2MB - keep them off the PE-feeding
                    # SP ring (starves xT loads); gpsimd absorbs them
                    nc.gpsimd.dma_start(
                        out=fused_d[drow:drow + nrows, :],
                        in_=g_sb[sbr:sbr + nrows, :],
                    )

            def emit_mask(t, eng=None):
                kmax = kmax_list[t]
                if kmax == 0:
                    return
                rows = min(P, L - t * P)
                m_sb = mpool.tile([P, L], mybir.dt.uint8, tag="m", bufs=6,
                                  name="m_sb")
                nc.vector.tensor_scalar(
                    out=m_sb[:, :kmax],
                    in0=iota_f[:, :kmax],
                    scalar1=k_sb[:, t:t + 1],
                    scalar2=None,
                    op0=mybir.AluOpType.is_lt,
                )
                (eng or wr_eng()).dma_start(
                    out=mask_d[t * P:t * P + rows, 0:kmax],
                    in_=m_sb[:rows, :kmax],
                )

            # Interleave the three streams so every DMA queue is fed from
            # t=0. Natural mask order measured fastest: size-reordered or
            # ring-split variants all disturbed the static schedule.
            steps = max(len(used_m), n_ttiles, 1)
            mask_emitted = 0
            for step in range(steps):
                if step < len(used_m):
                    emit_visual(used_m[step])
                if step < n_ttiles:
                    emit_text(step)
                target = ((step + 1) * n_mtiles + steps - 1) // steps
                while mask_emitted < min(target, n_mtiles):
                    emit_mask(mask_emitted)
                    mask_emitted += 1
            while mask_emitted < n_mtiles:
                emit_mask(mask_emitted)
                mask_emitted += 1

            # ---- zero-fill padded fused rows (ragged batch only) -----------
            if l < L:
                z_sb = cpool.tile([P, D], f32)
                nc.vector.memset(z_sb[:], 0.0)
                r = l
                while r < L:
                    nrows = min(P, L - r)
                    wr_eng().dma_start(
                        out=fused_d[r:r + nrows, :], in_=z_sb[:nrows, :]
                    )
                    r += nrows

    return _split_excess_waits(nc) if _SPLIT_WAITS else nc


# ---------------------------------------------------------------------------
# kernel() entry point
# ---------------------------------------------------------------------------

def _run_group(nc, in_maps):
    from concourse.bass_utils import run_bass_kernel_spmd

    global LAST_RESULTS
    res = run_bass_kernel_spmd(nc, in_maps, core_ids=list(range(len(in_maps))))
    LAST_RESULTS = res
    return res.results


# overridable for simulator-based testing
_EXECUTOR = _run_group


def kernel(**inputs):
    visual = np.ascontiguousarray(
        np.asarray(inputs["visual_features"], dtype=np.float32)
    )
    emb = np.ascontiguousarray(
        np.asarray(inputs["embedding_table"], dtype=np.float32)
    )
    W = np.ascontiguousarray(np.asarray(inputs["W_proj"], dtype=np.float32))
    bias = np.ascontiguousarray(
        np.asarray(inputs["b_proj"], dtype=np.float32).reshape(1, -1)
    )
    texts = np.asarray(inputs["texts"]).astype(np.int64)
    itid = int(np.asarray(inputs["image_token_id"]))

    B, S = texts.shape
    _, Nv, DV = visual.shape
    V, D = emb.shape
    assert DV % P == 0 and Nv % P == 0 and D % NSPLIT == 0

    idx_list, cid_list, vis_list = _plan_lists(texts, itid, Nv)
    L = max(len(x) for x in idx_list)
    n_mtiles = math.ceil(L / P)

    metas = [
        _sample_meta(texts[b], idx_list[b], cid_list[b], vis_list[b], S, L)
        for b in range(B)
    ]

    # Group samples by program structure (identical for the standard input).
    groups = {}
    for b, meta in enumerate(metas):
        kmax_list = tuple(
            int(meta["K"][t * P: min((t + 1) * P, L)].max())
            for t in range(n_mtiles)
        )
        key = (V, D, DV, Nv, L, meta["n_text"], meta["text_segs"],
               meta["vis_segs"], kmax_list, meta["l"])
        groups.setdefault(key, []).append(b)

    fused_out = np.zeros((B, L, D), np.float32)
    mask_out = np.zeros((B, 1, L, L), np.float32)

    for struct, bs in groups.items():
        nc = _build_nc(struct)
        n_text = struct[5]
        n_ttiles = math.ceil(n_text / P) if n_text else 0
        in_maps = []
        for b in bs:
            meta = metas[b]
            im = {
                "emb": emb,
                "xt": np.ascontiguousarray(visual[b].T),
                "w": W,
                "bias": bias,
                "ones": np.ones((1, P), np.float32),
                "kvec": np.ascontiguousarray(
                    np.pad(
                        meta["K"].astype(np.float32),
                        (0, n_mtiles * P - L),
                    ).reshape(n_mtiles, P).T
                ),
            }
            if n_ttiles:
                im["tok"] = np.ascontiguousarray(
                    np.pad(meta["tok_rows"], (0, n_ttiles * P - n_text))
                    .reshape(n_ttiles, P).T.astype(np.int32)
                )
            in_maps.append(im)
        results = _EXECUTOR(nc, in_maps)
        for b, res in zip(bs, results):
            fused_out[b] = res["fused"]
            mask_out[b, 0] = res["maskout"].astype(np.float32)

    return fused_out, mask_out
